# revision 1
# baseline (speedup 1.0000x reference)
"""Bidirectional Mamba block (nn_BiMamba) on 8 Trainium2 NeuronCores.

Sharding: core c in 0..7 -> (batch b = c>>2, direction dir = (c>>1)&1,
state-half sh = c&1).  Each core runs the full per-direction mamba pipeline
for its batch with the selective-scan state sharded over d_state (128 of 256
channels per core).  The scan itself uses the DVE tensor_tensor_scan
instruction over tiles of (s=128 partitions, t=1024 time) per d_inner
channel.  Partial scan outputs are AllReduce'd over state-half pairs; the
direction sum ((xf+xb)/2) is an AllReduce over the 4-core batch group with
out_proj pre-scaled by 0.25.  Direction flips and the site/cell reorder
between the two mamba layers are handled with host-fed gather indices so
the program is identical (SPMD) on all cores.
"""

import json
import math

import numpy as np

import concourse.bass as bass
import concourse.mybir as mybir
import concourse.tile as tile
from concourse.bass_utils import run_bass_kernel_spmd

AF = mybir.ActivationFunctionType
ALU = mybir.AluOpType
F32 = mybir.dt.float32

# problem dims (hardcoded per task contract)
B, NSITE, NCELL, DIM = 2, 128, 8, 32
DM, DI, DS, DR, DCONV = 2 * DIM, 4 * DIM, 256, 4, 4
L = NSITE * NCELL            # 1024
SL = DS // 2                 # 128 state channels per core
N_CORES = 8
EPS = 1e-5

# ---------------------------------------------------------------------------
# BIR post-processing: this walrus build accepts at most ONE sync wait per
# instruction; hoist excess waits onto standalone NoOp carriers.
# ---------------------------------------------------------------------------

def _split_waits(bir_json: bytes, maxw: int = 1) -> bytes:
    data = json.loads(bir_json)
    n = [0]

    def carrier(ins, waits):
        n[0] += 1
        return {
            "debug": ins.get("debug", 0),
            "engine": ins["engine"],
            "ins": [],
            "name": f"I-wsplit-{n[0]}",
            "opcode": "NoOp",
            "outs": [],
            "sync_info": {"on_update": [], "on_wait": waits},
        }

    for fn in data["functions"]:
        for blk in fn["blocks"]:
            out = []
            for ins in blk["instructions"]:
                si = ins.get("sync_info")
                if si and si.get("on_wait") and len(si["on_wait"]) > maxw:
                    waits = si["on_wait"]
                    extra, keep = waits[:-maxw], waits[-maxw:]
                    for i in range(0, len(extra), maxw):
                        out.append(carrier(ins, extra[i:i + maxw]))
                    si["on_wait"] = keep
                out.append(ins)
            blk["instructions"] = out
    return json.dumps(data).encode()


_orig_to_json_bytes = bass.Bass.to_json_bytes


def _patched_to_json_bytes(self, *a, **k):
    return _split_waits(_orig_to_json_bytes(self, *a, **k))


bass.Bass.to_json_bytes = _patched_to_json_bytes

# ---------------------------------------------------------------------------
# device program
# ---------------------------------------------------------------------------

def build_bass(nrep: int = 1, no_cc: bool = False):
    nc = bass.Bass(num_devices=N_CORES)
    f32 = F32

    def din(name, shape, dtype=f32):
        return nc.dram_tensor(name, shape, dtype, kind="ExternalInput")

    emb_T = din("emb_T", [3 * DIM, L])
    pos_T = din("pos_T", [3 * DIM, L])
    fccT = din("fccT", [3 * DIM, DM])
    fccb = din("fccb", [DM, 1])
    inprojT = din("inprojT", [DM, 2 * DI])
    convW = din("convW", [DI, DCONV])
    convB = din("convB", [DI, 1])
    xprojT = din("xprojT", [DI, DR + 2 * SL])   # [dtr | B_half | C_half]
    dtwT = din("dtwT", [DR, DI])
    ndtb = din("ndtb", [DI, 1])                 # minus dt_proj_b
    aposT = din("aposT", [SL, DI])              # +exp(A_log).T slice (s, d)
    dcol = din("dcol", [DI, 1])
    lng = din("lng", [DM, 1])
    lnb = din("lnb", [DM, 1])
    outprojT = din("outprojT", [DI, DM])        # pre-scaled by 0.25
    idx_rev = din("idx_rev", [128, L // 16], mybir.dt.uint16)
    idx2 = din("idx2", [128, L // 16], mybir.dt.uint16)

    out_h = nc.dram_tensor("out", [DM, L], f32, kind="ExternalOutput")

    with tile.TileContext(nc) as tc:
        with (
            tc.tile_pool(name="cst", bufs=1) as cst,
            tc.tile_pool(name="res", bufs=1) as res,
            tc.tile_pool(name="blk", bufs=1) as blk,
            tc.tile_pool(name="tr", bufs=1) as tr,
            tc.tile_pool(name="pp", bufs=4) as pp,
            tc.tile_pool(name="rw", bufs=4) as rw,
            tc.tile_pool(name="stg", bufs=3) as stg,
            tc.tile_pool(name="lp", bufs=3) as lp,
            tc.tile_pool(name="pmm", bufs=2, space="PSUM") as pmm,
            tc.tile_pool(name="pa", bufs=2, space="PSUM") as pa,
            tc.tile_pool(name="px", bufs=2, space="PSUM") as px,
            tc.tile_pool(name="py", bufs=1, space="PSUM") as py,
            tc.tile_pool(name="dram", bufs=2, space="DRAM") as dram,
        ):
            # ---- load constants / weights ----
            def load(t, shape, dtype=f32):
                s = cst.tile(shape, dtype, tag=t.name)
                nc.sync.dma_start(s[:], t[:])
                return s

            s_emb = load(emb_T, [3 * DIM, L])
            s_pos = load(pos_T, [3 * DIM, L])
            s_fccT = load(fccT, [3 * DIM, DM])
            s_fccb = load(fccb, [DM, 1])
            s_inpT = load(inprojT, [DM, 2 * DI])
            s_convW = load(convW, [DI, DCONV])
            s_convB = load(convB, [DI, 1])
            s_xprT = load(xprojT, [DI, DR + 2 * SL])
            s_dtwT = load(dtwT, [DR, DI])
            s_ndtb = load(ndtb, [DI, 1])
            s_aposT = load(aposT, [SL, DI])
            s_dcol = load(dcol, [DI, 1])
            s_lng = load(lng, [DM, 1])
            s_lnb = load(lnb, [DM, 1])
            s_outT = load(outprojT, [DI, DM])
            s_irev = load(idx_rev, [128, L // 16], mybir.dt.uint16)
            s_idx2 = load(idx2, [128, L // 16], mybir.dt.uint16)

            ones_row = cst.tile([1, DI], mybir.dt.bfloat16, tag="ones_row")
            nc.vector.memset(ones_row[:], 1.0)
            ones_col = cst.tile([SL, 1], f32, tag="ones_col")
            nc.vector.memset(ones_col[:], 1.0)
            onesdm_col = cst.tile([DM, 1], f32, tag="onesdm_col")
            nc.vector.memset(onesdm_col[:], 1.0)
            onesdm_row = cst.tile([1, DM], f32, tag="onesdm_row")
            nc.vector.memset(onesdm_row[:], 1.0)
            eps_t = cst.tile([1, 1], f32, tag="eps_t")
            nc.vector.memset(eps_t[:], EPS)
            zo = cst.tile([SL, 2 * DI], mybir.dt.bfloat16, tag="zo")
            nc.vector.memset(zo[:], 0.0)
            nc.vector.memset(zo[:, DI:DI + 1], 1.0)

            H = 512  # matmul free-dim chunk
            NH = L // H

            def halves():
                return [(h, h * H, (h + 1) * H) for h in range(NH)]

            def mamba_block(u, base_canon, tag):
                """u: (DM, L) sbuf tile, local-order input sequence.
                base_canon: (DM, L) residual base in canonical order.
                Returns h_next (DM, L) canonical order."""
                # ---- in_proj ----
                xcpad = blk.tile([DI, DCONV - 1 + L], f32, tag="xcpad")
                nc.vector.memset(xcpad[:, 0:DCONV - 1], 0.0)
                sigz = tr.tile([DI, L], f32, tag="sigz")
                zg = blk.tile([DI, L], f32, tag="zg")
                for _, a, b2 in halves():
                    p = pmm.tile([DI, H], f32, tag="pmm")
                    nc.tensor.matmul(p[:], s_inpT[:, 0:DI], u[0:DM, a:b2],
                                     start=True, stop=True)
                    nc.scalar.activation(xcpad[:, DCONV - 1 + a:DCONV - 1 + b2],
                                         p[:], AF.Copy)
                for _, a, b2 in halves():
                    p = pmm.tile([DI, H], f32, tag="pmm")
                    nc.tensor.matmul(p[:], s_inpT[:, DI:2 * DI], u[0:DM, a:b2],
                                     start=True, stop=True)
                    nc.scalar.activation(sigz[:, a:b2], p[:], AF.Sigmoid)
                    nc.vector.tensor_mul(zg[:, a:b2], sigz[:, a:b2], p[:])

                # ---- depthwise causal conv + silu ----
                cv = tr.tile([DI, L], f32, tag="cv")
                cv2 = tr.tile([DI, L], f32, tag="cv2")
                nc.vector.tensor_scalar(cv[:], xcpad[:, 0:L], s_convW[:, 0:1],
                                        s_convB[:, 0:1], ALU.mult, ALU.add)
                nc.vector.scalar_tensor_tensor(cv2[:], xcpad[:, 1:1 + L],
                                               s_convW[:, 1:2], cv[:],
                                               ALU.mult, ALU.add)
                nc.vector.scalar_tensor_tensor(cv[:], xcpad[:, 2:2 + L],
                                               s_convW[:, 2:3], cv2[:],
                                               ALU.mult, ALU.add)
                nc.vector.scalar_tensor_tensor(cv2[:], xcpad[:, 3:3 + L],
                                               s_convW[:, 3:4], cv[:],
                                               ALU.mult, ALU.add)
                sigc = tr.tile([DI, L], f32, tag="sigc")
                nc.scalar.activation(sigc[:], cv2[:], AF.Sigmoid)
                xc = blk.tile([DI, L], f32, tag="xc")
                nc.vector.tensor_mul(xc[:], cv2[:], sigc[:])

                # ---- x_proj ----
                dtr = tr.tile([DR, L], f32, tag="dtr")
                BT = blk.tile([SL, L], f32, tag="BT")     # negated
                CT = blk.tile([SL, L], mybir.dt.bfloat16, tag="CT")
                for _, a, b2 in halves():
                    p = pmm.tile([DI, H], f32, tag="pmm")
                    nc.tensor.matmul(p[0:DR, :], s_xprT[:, 0:DR], xc[:, a:b2],
                                     start=True, stop=True)
                    nc.scalar.activation(dtr[:, a:b2], p[0:DR, :], AF.Copy)
                BTb = tr.tile([SL, L], mybir.dt.bfloat16, tag="BTb")
                for _, a, b2 in halves():
                    p = pmm.tile([DI, H], f32, tag="pmm")
                    nc.tensor.matmul(p[0:SL, :], s_xprT[:, DR:DR + SL],
                                     xc[:, a:b2], start=True, stop=True)
                    # negate B so that b = dtn * xc * (-B) == dt*xc*B
                    nc.scalar.activation(BT[:, a:b2], p[0:SL, :], AF.Copy,
                                         scale=-1.0)
                    nc.scalar.activation(BTb[:, a:b2], p[0:SL, :], AF.Copy,
                                         scale=-1.0)
                for _, a, b2 in halves():
                    p = pmm.tile([DI, H], f32, tag="pmm")
                    nc.tensor.matmul(p[0:SL, :], s_xprT[:, DR + SL:DR + 2 * SL],
                                     xc[:, a:b2], start=True, stop=True)
                    nc.scalar.activation(CT[:, a:b2], p[0:SL, :], AF.Copy)

                # ---- dt (log-sigmoid form): dtn = log(sigmoid(-(lin+b)))
                #      = -softplus(lin + b) = -dt ----
                sdt = tr.tile([DI, L], f32, tag="sdt")
                for _, a, b2 in halves():
                    p = pmm.tile([DI, H], f32, tag="pmm")
                    nc.tensor.matmul(p[:], s_dtwT[:], dtr[:, a:b2],
                                     start=True, stop=True)
                    nc.scalar.activation(sdt[:, a:b2], p[:], AF.Sigmoid,
                                         scale=-1.0, bias=s_ndtb[:, 0:1])
                dtn = blk.tile([DI, L], f32, tag="dtn")
                nc.scalar.activation(dtn[:], sdt[:], AF.Ln)
                dthi = blk.tile([DI, L], mybir.dt.bfloat16, tag="dthi")
                nc.scalar.activation(dthi[:], dtn[:], AF.Copy)
                dtxn = blk.tile([DI, L], mybir.dt.bfloat16, tag="dtxn")
                nc.vector.tensor_mul(dtxn[:], dtn[:], xc[:])

                # ---- selective scan over d channels ----
                ys_ps = py.tile([SL, L], f32, tag="ys")
                bf16 = mybir.dt.bfloat16
                for rep in range(nrep):
                    for g4 in range(DI // 4):
                        st_hi = stg.tile([1, 4 * L], bf16, tag="st_hi")
                        nc.sync.dma_start(st_hi[:], dthi[4 * g4:4 * g4 + 4, :])
                        st_dx = stg.tile([1, 4 * L], bf16, tag="st_dx")
                        nc.sync.dma_start(st_dx[:], dtxn[4 * g4:4 * g4 + 4, :])
                        for j4 in range(4):
                            d = 4 * g4 + j4
                            a_d = lp.tile([SL, L], bf16, tag="a_d")
                            b_d = lp.tile([SL, L], bf16, tag="b_d")
                            if d % 3 != 0:
                                dxb = lp.tile([SL, L], bf16, tag="dxb")
                            for _, a, b2 in halves():
                                pdt = pa.tile([SL, H], f32, tag="pdt")
                                nc.tensor.matmul(
                                    pdt[:], ones_row[:],
                                    st_hi[0:1, j4 * L + a:j4 * L + b2],
                                    start=True, stop=True)
                                nc.scalar.activation(
                                    a_d[:, a:b2], pdt[:], AF.Exp,
                                    scale=s_aposT[:, d:d + 1])
                                pdx = px.tile([SL, H], f32, tag="pdx")
                                nc.tensor.matmul(
                                    pdx[:], ones_row[:],
                                    st_dx[0:1, j4 * L + a:j4 * L + b2],
                                    start=True, stop=True)
                                if d % 3 == 0:
                                    # path A: DVE multiplies from PSUM (1x)
                                    nc.vector.tensor_mul(b_d[:, a:b2],
                                                         BT[:, a:b2], pdx[:])
                                else:
                                    # path B: ACT evacuates to bf16, DVE
                                    # multiplies at 2x (both operands bf16)
                                    nc.scalar.activation(dxb[:, a:b2], pdx[:],
                                                         AF.Copy)
                                    nc.vector.tensor_mul(b_d[:, a:b2],
                                                         BTb[:, a:b2],
                                                         dxb[:, a:b2])
                            h_d = lp.tile([SL, L], bf16, tag="h_d")
                            nc.vector.tensor_tensor_scan(
                                h_d[:], a_d[:], b_d[:], 0.0,
                                ALU.mult, ALU.add)
                            m_d = lp.tile([SL, L], mybir.dt.bfloat16, tag="m_d")
                            nc.gpsimd.tensor_mul(m_d[:], h_d[:], CT[:])
                            for _, a, b2 in halves():
                                nc.tensor.matmul(
                                    ys_ps[:, a:b2], zo[:, DI - d:2 * DI - d],
                                    m_d[:, a:b2], start=(d == 0 and rep == 0),
                                    stop=(d == DI - 1 and rep == nrep - 1),
                                    skip_group_check=True)
                ys = pp.tile([SL, L], f32, tag="pp")
                nc.scalar.activation(ys[:], ys_ps[:], AF.Copy)

                # ---- AllReduce partial ys over state-half pairs ----
                cc1_in = dram.tile([SL, L], f32, tag="cc1i")
                cc1_out = dram.tile([SL, L], f32, tag="cc1o")
                nc.gpsimd.dma_start(cc1_in[:], ys[:])
                if no_cc:
                    nc.gpsimd.dma_start(cc1_out[:], cc1_in[:])
                else:
                    nc.gpsimd.collective_compute(
                        "AllReduce", ALU.add,
                        replica_groups=[[0, 1], [2, 3], [4, 5], [6, 7]],
                        ins=[cc1_in.opt()], outs=[cc1_out.opt()])
                ysf = pp.tile([SL, L], f32, tag="pp")
                nc.gpsimd.dma_start(ysf[:], cc1_out[:])

                # ---- gate + out_proj ----
                g1 = pp.tile([DI, L], f32, tag="pp")
                nc.vector.scalar_tensor_tensor(g1[:], xc[:], s_dcol[:, 0:1],
                                               ysf[:], ALU.mult, ALU.add)
                gated = pp.tile([DI, L], f32, tag="pp")
                nc.vector.tensor_mul(gated[:], g1[:], zg[:])
                yo = pp.tile([128, L], f32, tag="pp")
                for _, a, b2 in halves():
                    p = pmm.tile([DI, H], f32, tag="pmm")
                    nc.tensor.matmul(p[0:DM, :], s_outT[:], gated[:, a:b2],
                                     start=True, stop=True)
                    nc.scalar.activation(yo[0:DM, a:b2], p[0:DM, :], AF.Copy)
                yc = pp.tile([128, L], f32, tag="pp")
                nc.gpsimd.indirect_copy(yc[:], yo[:], s_irev[:], True)

                # ---- AllReduce over the 4-core batch group: 2*(yf+yb),
                #      out_proj pre-scaled 0.25 -> (yf+yb)/2 ----
                cc2_in = dram.tile([DM, L], f32, tag="cc2i")
                cc2_out = dram.tile([DM, L], f32, tag="cc2o")
                nc.gpsimd.dma_start(cc2_in[:], yc[0:DM, :])
                if no_cc:
                    nc.gpsimd.dma_start(cc2_out[:], cc2_in[:])
                else:
                    nc.gpsimd.collective_compute(
                        "AllReduce", ALU.add,
                        replica_groups=[[0, 1, 2, 3], [4, 5, 6, 7]],
                        ins=[cc2_in.opt()], outs=[cc2_out.opt()])
                ysum = pp.tile([DM, L], f32, tag="pp")
                nc.gpsimd.dma_start(ysum[:], cc2_out[:])

                # ---- residual + layernorm (canonical order) ----
                rsd = pp.tile([DM, L], f32, tag="pp")
                nc.vector.tensor_add(rsd[:], base_canon[0:DM, :], ysum[:])
                mu = rw.tile([1, L], f32, tag="rw")
                sq = pp.tile([DM, L], f32, tag="pp")
                nc.vector.tensor_mul(sq[:], rsd[:], rsd[:])
                lnv = rw.tile([1, L], f32, tag="rw")
                rstd = rw.tile([1, L], f32, tag="rw")
                cen = pp.tile([DM, L], f32, tag="pp")
                for _, a, b2 in halves():
                    p = pmm.tile([DI, H], f32, tag="pmm")
                    nc.tensor.matmul(p[0:1, :], onesdm_col[:], rsd[:, a:b2],
                                     start=True, stop=True)
                    nc.scalar.activation(mu[0:1, a:b2], p[0:1, :], AF.Copy,
                                         scale=1.0 / DM)
                for _, a, b2 in halves():
                    p = pmm.tile([DI, H], f32, tag="pmm")
                    nc.tensor.matmul(p[0:DM, :], onesdm_row[:], mu[0:1, a:b2],
                                     start=True, stop=True)
                    nc.vector.tensor_sub(cen[:, a:b2], rsd[:, a:b2],
                                         p[0:DM, :])
                # var = mean(rsd^2) - mu^2 ... use E[x^2] - mu^2 form:
                for _, a, b2 in halves():
                    p = pmm.tile([DI, H], f32, tag="pmm")
                    nc.tensor.matmul(p[0:1, :], onesdm_col[:], sq[:, a:b2],
                                     start=True, stop=True)
                    # lnv = ln(E[x^2] - mu^2 + eps) needs mu^2; instead
                    # compute via centered square below. Copy sum here.
                    nc.scalar.activation(lnv[0:1, a:b2], p[0:1, :], AF.Copy,
                                         scale=1.0 / DM)
                # rstd = exp(-0.5*ln(var+eps)), var = E[x^2]-mu^2
                musq = rw.tile([1, L], f32, tag="rw")
                nc.vector.tensor_mul(musq[:], mu[:], mu[:])
                varv = rw.tile([1, L], f32, tag="rw")
                nc.vector.tensor_sub(varv[:], lnv[:], musq[:])
                lvar = rw.tile([1, L], f32, tag="rw")
                nc.scalar.activation(lvar[:], varv[:], AF.Ln, bias=eps_t[0:1, 0:1])
                nc.scalar.activation(rstd[:], lvar[:], AF.Exp, scale=-0.5)
                nrm = pp.tile([DM, L], f32, tag="pp")
                for _, a, b2 in halves():
                    p = pmm.tile([DI, H], f32, tag="pmm")
                    nc.tensor.matmul(p[0:DM, :], onesdm_row[:],
                                     rstd[0:1, a:b2], start=True, stop=True)
                    nc.vector.tensor_mul(nrm[:, a:b2], cen[:, a:b2],
                                         p[0:DM, :])
                hn = pp.tile([128, L], f32, tag="pp")
                nc.vector.tensor_scalar(hn[0:DM, :], nrm[:], s_lng[:, 0:1],
                                        s_lnb[:, 0:1], ALU.mult, ALU.add)
                return hn

            # ---- block 1 ----
            ep = res.tile([3 * DIM, L], f32, tag="ep")
            nc.vector.tensor_add(ep[:], s_emb[:], s_pos[:])
            h0 = res.tile([128, L], f32, tag="h0")
            for hh in range(NH):
                a, b2 = hh * H, (hh + 1) * H
                p = pmm.tile([DI, H], f32, tag="pmm")
                nc.tensor.matmul(p[0:DM, :], s_fccT[:], ep[:, a:b2],
                                 start=True, stop=True)
                nc.vector.tensor_scalar(h0[0:DM, a:b2], p[0:DM, :],
                                        s_fccb[:, 0:1], 0.0, ALU.add, ALU.max)
            h0cw = res.tile([128, L], f32, tag="h0cw")
            nc.gpsimd.indirect_copy(h0cw[:], h0[:], s_irev[:], True)

            h2 = mamba_block(h0, h0cw, "b1")

            # ---- transition: site-major -> cell-major ----
            h2t2 = res.tile([DM, L], f32, tag="h2t2")
            nc.vector.tensor_copy(
                h2t2[:].rearrange("p (c s) -> p c s", s=NSITE),
                h2[0:DM, :].rearrange("p (s c) -> p s c", c=NCELL).transpose([0, 2, 1]))
            u2 = res.tile([128, L], f32, tag="u2")
            nc.gpsimd.indirect_copy(u2[:], h2[:], s_idx2[:], True)

            h3 = mamba_block(u2, h2t2, "b2")
            nc.sync.dma_start(out_h[:], h3[0:DM, :])

    return nc


# ---------------------------------------------------------------------------
# host side
# ---------------------------------------------------------------------------

def _pos_enc(D, Hh, W):
    pe = np.zeros((D, Hh, W), np.float32)
    dm = D // 2
    div = np.exp(np.arange(0, dm, 2, dtype=np.float32) * -(math.log(10000.0) / dm))
    pw = np.arange(W, dtype=np.float32)[:, None]
    ph = np.arange(Hh, dtype=np.float32)[:, None]
    pe[0:dm:2] = np.broadcast_to(np.sin(pw * div).T[:, None, :], (dm // 2, Hh, W))
    pe[1:dm:2] = np.broadcast_to(np.cos(pw * div).T[:, None, :], (dm // 2, Hh, W))
    pe[dm::2] = np.broadcast_to(np.sin(ph * div).T[:, :, None], (dm // 2, Hh, W))
    pe[dm + 1::2] = np.broadcast_to(np.cos(ph * div).T[:, :, None], (dm // 2, Hh, W))
    return pe.transpose(1, 2, 0)  # (H, W, D)


def _wrap_idx(vec):
    """indirect_copy index layout: index j lives at (partition j%16,
    slot j//16), replicated for each 16-partition group."""
    w = np.zeros((128, L // 16), np.uint16)
    blkv = vec.reshape(L // 16, 16).T.astype(np.uint16)
    for g in range(128 // 16):
        w[g * 16:(g + 1) * 16, :] = blkv
    return w


def make_in_maps(inputs):
    x = np.asarray(inputs["x"], np.float32)
    y = np.asarray(inputs["y"]).astype(np.int64)
    ci = np.asarray(inputs["cell_indices"]).astype(np.int64)
    cellEB = np.asarray(inputs["cellEB"], np.float32)
    CpGEB = np.asarray(inputs["CpGEB"], np.float32)
    fcc_w = np.asarray(inputs["fcc_w"], np.float32)
    fcc_b = np.asarray(inputs["fcc_b"], np.float32)
    ln_g = np.asarray(inputs["ln_g"], np.float32)
    ln_b = np.asarray(inputs["ln_b"], np.float32)
    in_proj_w = np.asarray(inputs["in_proj_w"], np.float32)
    conv_w = np.asarray(inputs["conv_w"], np.float32)
    conv_b = np.asarray(inputs["conv_b"], np.float32)
    x_proj_w = np.asarray(inputs["x_proj_w"], np.float32)
    dt_proj_w = np.asarray(inputs["dt_proj_w"], np.float32)
    dt_proj_b = np.asarray(inputs["dt_proj_b"], np.float32)
    A_log = np.asarray(inputs["A_log"], np.float32)
    D_param = np.asarray(inputs["D_param"], np.float32)
    out_proj_w = np.asarray(inputs["out_proj_w"], np.float32)

    pos = _pos_enc(3 * DIM, NSITE, NCELL)          # (site, cell, 96)
    pos_t1 = pos.reshape(L, 3 * DIM)

    # embedding gather + concat (site-major t1 ordering)
    emb = np.concatenate([
        CpGEB[y],                                   # (B, site, cell, 32)
        np.broadcast_to(cellEB[ci][:, None], (B, NSITE, NCELL, DIM)),
        np.broadcast_to(x[:, :, None, :], (B, NSITE, NCELL, DIM)),
    ], axis=-1).reshape(B, L, 3 * DIM)

    Apos = np.exp(A_log)                            # |A| = -A, (DI, DS)
    XT = x_proj_w.T.copy()                          # (DI, 516)

    idx_id = np.arange(L, dtype=np.int64)
    idx_rv = idx_id[::-1].copy()
    # t2 permutation: u2[v] = h2_canon[perm0[v]], v = c*NSITE + s
    v = np.arange(L)
    c_, s_ = v // NSITE, v % NSITE
    perm0 = s_ * NCELL + c_

    in_maps = []
    for core in range(N_CORES):
        b = core >> 2
        dirb = (core >> 1) & 1
        sh = core & 1
        e = emb[b] if dirb == 0 else emb[b][::-1]
        p1 = pos_t1 if dirb == 0 else pos_t1[::-1]
        m = {
            "emb_T": np.ascontiguousarray(e.T),
            "pos_T": np.ascontiguousarray(p1.T),
            "fccT": np.ascontiguousarray(fcc_w.T),
            "fccb": fcc_b.reshape(DM, 1),
            "inprojT": np.ascontiguousarray(in_proj_w.T),
            "convW": np.ascontiguousarray(conv_w[:, 0, :]),
            "convB": conv_b.reshape(DI, 1),
            "xprojT": np.ascontiguousarray(np.hstack([
                XT[:, 0:DR],
                XT[:, DR + sh * SL:DR + (sh + 1) * SL],
                XT[:, DR + DS + sh * SL:DR + DS + (sh + 1) * SL]])),
            "dtwT": np.ascontiguousarray(dt_proj_w.T),
            "ndtb": (-dt_proj_b).reshape(DI, 1),
            "aposT": np.ascontiguousarray(Apos[:, sh * SL:(sh + 1) * SL].T),
            "dcol": D_param.reshape(DI, 1),
            "lng": ln_g.reshape(DM, 1),
            "lnb": ln_b.reshape(DM, 1),
            "outprojT": np.ascontiguousarray(out_proj_w.T) * 0.25,
            "idx_rev": _wrap_idx(idx_id if dirb == 0 else idx_rv),
            "idx2": _wrap_idx(perm0 if dirb == 0 else perm0[::-1]),
        }
        in_maps.append(m)
    return in_maps


def postprocess(results):
    out = np.zeros((B, NSITE, NCELL, DM), np.float32)
    for b, core in ((0, 0), (1, 4)):
        h3 = results[core]["out"]                   # (DM, L) t2-canonical
        seq = h3.T.reshape(NCELL, NSITE, DM)        # v = c*NSITE + s
        out[b] = seq.transpose(1, 0, 2)
    return out



# ---------------------------------------------------------------------------
# cached PJRT runner (built once per process; repeat kernel() calls are fast)
# ---------------------------------------------------------------------------
import time

import jax
from jax.sharding import Mesh, PartitionSpec
from jax.experimental.shard_map import shard_map

from concourse.bass2jax import _bass_exec_p, install_neuronx_cc_hook, partition_id_tensor


class Runner:
    def __init__(self, nc, in_maps, n_cores=8):
        install_neuronx_cc_hook()
        self.n_cores = n_cores
        partition_name = nc.partition_id_tensor.name if nc.partition_id_tensor else None
        in_names, out_names, out_avals, zero_outs = [], [], [], []
        for alloc in nc.m.functions[0].allocations:
            if not isinstance(alloc, mybir.MemoryLocationSet):
                continue
            name = alloc.memorylocations[0].name
            if alloc.kind == "ExternalInput":
                if name != partition_name:
                    in_names.append(name)
            elif alloc.kind == "ExternalOutput":
                out_names.append(name)
                shape = tuple(alloc.tensor_shape)
                dtype = mybir.dt.np(alloc.dtype)
                out_avals.append(jax.core.ShapedArray(shape, dtype))
                zero_outs.append(np.zeros(shape, dtype))
        n_params = len(in_names)
        n_outs = len(out_avals)
        all_in_names = list(in_names) + out_names
        if partition_name is not None:
            all_in_names.append(partition_name)
        donate = tuple(range(n_params, n_params + n_outs))

        def _body(*args):
            operands = list(args)
            if partition_name is not None:
                operands.append(partition_id_tensor())
            outs = _bass_exec_p.bind(
                *operands,
                out_avals=tuple(out_avals),
                in_names=tuple(all_in_names),
                out_names=tuple(out_names),
                lowering_input_output_aliases=(),
                sim_require_finite=True,
                sim_require_nnan=True,
                nc=nc,
            )
            return tuple(outs)

        devices = jax.devices()[:n_cores]
        mesh = Mesh(np.asarray(devices), ("core",))
        in_specs = (PartitionSpec("core"),) * (n_params + n_outs)
        out_specs = (PartitionSpec("core"),) * n_outs
        self.f = jax.jit(
            shard_map(_body, mesh=mesh, in_specs=in_specs,
                      out_specs=out_specs, check_rep=False),
            donate_argnums=donate, keep_unused=True)
        self.in_names = in_names
        self.n_params = n_params
        self.sharding = jax.sharding.NamedSharding(mesh, PartitionSpec("core"))
        self.set_inputs(in_maps)
        zshapes = [(n_cores * z.shape[0], *z.shape[1:]) for z in zero_outs]
        zdt = [z.dtype for z in zero_outs]

        def _mkzeros():
            return tuple(jax.numpy.zeros(s, d) for s, d in zip(zshapes, zdt))

        self.mkzeros = jax.jit(_mkzeros, out_shardings=(self.sharding,) * n_outs)
        self.out_names = out_names
        self.out_avals = out_avals

    def set_inputs(self, in_maps):
        per_core = [[np.asarray(m[n]) for n in self.in_names] for m in in_maps]
        concat_in = [
            np.concatenate([per_core[c][i] for c in range(self.n_cores)], axis=0)
            for i in range(self.n_params)
        ]
        self.inputs_dev = [jax.device_put(a, self.sharding) for a in concat_in]

    def run(self):
        z = self.mkzeros()
        jax.block_until_ready(z)
        t0 = time.time()
        outs = self.f(*self.inputs_dev, *z)
        jax.block_until_ready(outs)
        dt = time.time() - t0
        return outs, dt

    def results(self, outs):
        res = []
        for c in range(self.n_cores):
            m = {}
            for i, name in enumerate(self.out_names):
                a = np.asarray(outs[i])
                m[name] = a.reshape(self.n_cores, *self.out_avals[i].shape)[c]
            res.append(m)
        return res

    def bench(self, warmup=2, iters=12):
        for _ in range(warmup):
            self.run()
        ts = []
        for _ in range(iters):
            _, dt = self.run()
            ts.append(dt)
        ts.sort()
        return ts[len(ts) // 2], ts[0]


_cache = {}


def _get_nc(nrep=1):
    if nrep not in _cache:
        _cache[nrep] = build_bass(nrep)
    return _cache[nrep]


_runner_cache = {}


def get_runner(inputs, nrep=1):
    key = nrep
    if key not in _runner_cache:
        _runner_cache[key] = Runner(_get_nc(nrep), make_in_maps(inputs), N_CORES)
    return _runner_cache[key]


def kernel(**inputs) -> np.ndarray:
    r = get_runner(inputs, 1)
    # refresh device inputs in case the caller passes different data
    in_maps = make_in_maps(inputs)
    r.set_inputs(in_maps)
    outs, _ = r.run()
    return postprocess(r.results(outs))



# revision 11
# speedup vs baseline: 1.0870x; 1.0870x over previous
"""Bidirectional Mamba block (nn_BiMamba) on 8 Trainium2 NeuronCores.

Sharding: core c = b*4 + dir*2 + dh -> (batch b, direction dir, d_inner-half
dh).  Host permutes the d_inner axis per core so the core's own 64 channels
are rows 0:64 of every phase tensor.  The selective scan runs on
"supertiles" [128, L]: partition p = 16-state-group x 8-channel-group
(p = s_local*8 + d_local), so one PE broadcast + one ACT exp serves 8
channels at once.  B/C are materialized replicated (x_proj with
host-replicated lhsT columns) so the scan-input multiply runs at DVE 2x.
Each core owns 64 channels x all 256 states -> the state reduction is
core-local and only ONE AllReduce (over the 4-core batch group: 2 dirs x 2
halves) is needed per mamba layer, with out_proj pre-scaled by 0.5.
Direction flips / layer reorders use host-fed gather indices (SPMD).
"""

import json
import math

import numpy as np
from ml_dtypes import bfloat16

import concourse.bass as bass
import concourse.mybir as mybir
import concourse.tile as tile

AF = mybir.ActivationFunctionType
ALU = mybir.AluOpType
F32 = mybir.dt.float32
BF16 = mybir.dt.bfloat16

# problem dims (hardcoded per task contract)
B, NSITE, NCELL, DIM = 2, 128, 8, 32
DM, DI, DS, DR, DCONV = 2 * DIM, 4 * DIM, 256, 4, 4
L = NSITE * NCELL            # 1024
DH = DI // 2                 # 64 channels per core
NG = DH // 8                 # 8 channel-groups of 8
NJ = DS // 16                # 16 state-groups of 16
N_CORES = 8
EPS = 1e-5
H = 512                      # matmul free-dim chunk (PSUM bank)

# per-(g,j): which engine does the b-multiply / m-multiply.
# 'P' = Pool, 'V' = DVE.  Tuned for engine balance (DVE also runs scans).
POOL_B = {0, 3, 6, 7, 9, 12, 15}       # bmul on Pool for these j
POOL_M = {1, 4, 5, 11, 13, 14}         # mmul on Pool for these j
# -> per g: Pool 13 tiles, DVE 16 scans + 9 bmul + 10 mmul


# ---------------------------------------------------------------------------
# BIR post-processing: this walrus build accepts at most ONE sync wait per
# instruction; hoist excess waits onto standalone NoOp carriers.
# ---------------------------------------------------------------------------

def _split_waits(bir_json: bytes, maxw: int = 1) -> bytes:
    data = json.loads(bir_json)
    n = [0]

    def carrier(ins, waits):
        n[0] += 1
        return {
            "debug": ins.get("debug", 0),
            "engine": ins["engine"],
            "ins": [],
            "name": f"I-wsplit-{n[0]}",
            "opcode": "NoOp",
            "outs": [],
            "sync_info": {"on_update": [], "on_wait": waits},
        }

    for fn in data["functions"]:
        for blk in fn["blocks"]:
            out = []
            for ins in blk["instructions"]:
                si = ins.get("sync_info")
                if si and si.get("on_wait") and len(si["on_wait"]) > maxw:
                    waits = si["on_wait"]
                    extra, keep = waits[:-maxw], waits[-maxw:]
                    for i in range(0, len(extra), maxw):
                        out.append(carrier(ins, extra[i:i + maxw]))
                    si["on_wait"] = keep
                out.append(ins)
            blk["instructions"] = out
    return json.dumps(data).encode()


_orig_to_json_bytes = bass.Bass.to_json_bytes


def _patched_to_json_bytes(self, *a, **k):
    return _split_waits(_orig_to_json_bytes(self, *a, **k))


bass.Bass.to_json_bytes = _patched_to_json_bytes

# ---------------------------------------------------------------------------
# device program
# ---------------------------------------------------------------------------

def build_bass(nrep: int = 1, no_cc: bool = False):
    nc = bass.Bass(num_devices=N_CORES)
    f32 = F32

    def din(name, shape, dtype=f32):
        return nc.dram_tensor(name, shape, dtype, kind="ExternalInput")

    embpos_T = din("embpos_T", [3 * DIM, L])
    fccT = din("fccT", [3 * DIM, DM])
    fccb = din("fccb", [DM, 1])
    inprojT = din("inprojT", [DM, 2 * DI])
    convW = din("convW", [DI, DCONV])
    convB = din("convB", [DI, 1])
    xprojDtrT = din("xprojDtrT", [DI, DR], BF16)
    xprojRepB = din("xprojRepB", [DI, NJ * 128], BF16)
    xprojRepC = din("xprojRepC", [DI, NJ * 128], BF16)
    dtwT = din("dtwT", [DR, DI])
    dtb = din("dtb", [DI, 1])            # -dt_proj_b
    aposR = din("aposR", [128, 128])       # col g*16+j: -|A| for (s,d) lanes
    dcol = din("dcol", [DH, 1])
    lng = din("lng", [DM, 1])
    lnb = din("lnb", [DM, 1])
    outT = din("outT", [DH, DM], BF16)     # pre-scaled by 0.5
    selB = din("selB", [8, 128], BF16)     # d-broadcast selector
    Zsel = din("Zsel", [128, 128], BF16)   # reduce row-placement selector
    idx_rev = din("idx_rev", [128, L // 16], mybir.dt.uint16)
    idx2 = din("idx2", [128, L // 16], mybir.dt.uint16)

    out_h = nc.dram_tensor("out", [DM, L], f32, kind="ExternalOutput")

    with tile.TileContext(nc) as tc:
        with (
            tc.tile_pool(name="cst", bufs=1) as cst,
            tc.tile_pool(name="res", bufs=1) as res,
            tc.tile_pool(name="blk", bufs=1) as blk,
            tc.tile_pool(name="rep", bufs=1) as rep,
            tc.tile_pool(name="tr", bufs=1) as tr,
            tc.tile_pool(name="pp", bufs=3) as pp,
            tc.tile_pool(name="hp", bufs=2) as hp,
            tc.tile_pool(name="gp", bufs=2) as gp,
            tc.tile_pool(name="rw", bufs=3) as rw,
            tc.tile_pool(name="stg", bufs=2) as stg,
            tc.tile_pool(name="lp", bufs=3) as lp,
            tc.tile_pool(name="pa", bufs=2, space="PSUM") as pa,
            tc.tile_pool(name="px", bufs=2, space="PSUM") as px,
            tc.tile_pool(name="py", bufs=1, space="PSUM") as py,
            tc.tile_pool(name="dram", bufs=2, space="DRAM") as dram,
        ):
            def load(t, shape, dtype=f32):
                s = cst.tile(shape, dtype, tag=t.name)
                nc.sync.dma_start(s[:], t[:])
                return s

            s_embpos = load(embpos_T, [3 * DIM, L])
            s_fccT = load(fccT, [3 * DIM, DM])
            s_fccb = load(fccb, [DM, 1])
            s_inpT = load(inprojT, [DM, 2 * DI])
            s_convW = load(convW, [DI, DCONV])
            s_convB = load(convB, [DI, 1])
            s_xprDtr = load(xprojDtrT, [DI, DR], BF16)
            s_xprB = load(xprojRepB, [DI, NJ * 128], BF16)
            s_xprC = load(xprojRepC, [DI, NJ * 128], BF16)
            s_dtwT = load(dtwT, [DR, DI])
            s_dtb = load(dtb, [DI, 1])
            s_aposR = load(aposR, [128, 128])
            s_dcol = load(dcol, [DH, 1])
            s_lng = load(lng, [DM, 1])
            s_lnb = load(lnb, [DM, 1])
            s_outT = load(outT, [DH, DM], BF16)
            s_selB = load(selB, [8, 128], BF16)
            s_Z = load(Zsel, [128, 128], BF16)
            s_irev = load(idx_rev, [128, L // 16], mybir.dt.uint16)
            s_idx2 = load(idx2, [128, L // 16], mybir.dt.uint16)

            onesdm_col = cst.tile([DM, 1], f32, tag="onesdm_col")
            nc.vector.memset(onesdm_col[:], 1.0)
            onesdm_row = cst.tile([1, DM], f32, tag="onesdm_row")
            nc.vector.memset(onesdm_row[:], 1.0)
            eps_t = cst.tile([1, 1], f32, tag="eps_t")
            nc.vector.memset(eps_t[:], EPS)

            def halves():
                return [(h0, h0 * H, (h0 + 1) * H) for h0 in range(L // H)]

            def mamba_block(u, base_canon, tag):
                """u: (128, L) f32 tile, rows 0:DM valid, local-order input.
                base_canon: residual base in canonical order (rows 0:DM).
                Returns h_next (128, L) canonical order (rows 0:DM)."""
                # ---- in_proj ----
                xcpad = blk.tile([DI, DCONV - 1 + L], f32, tag="xcpad")
                nc.vector.memset(xcpad[:, 0:DCONV - 1], 0.0)
                zg = blk.tile([DH, L], f32, tag="zg")
                for _, a, b2 in halves():
                    p = px.tile([DI, H], f32, tag="px")
                    nc.tensor.matmul(p[:], s_inpT[:, 0:DI], u[0:DM, a:b2],
                                     start=True, stop=True)
                    nc.scalar.activation(xcpad[:, DCONV - 1 + a:DCONV - 1 + b2],
                                         p[:], AF.Copy)
                for _, a, b2 in halves():
                    p = px.tile([DI, H], f32, tag="px")
                    nc.tensor.matmul(p[0:DH, :], s_inpT[:, DI:DI + DH],
                                     u[0:DM, a:b2], start=True, stop=True)
                    nc.scalar.activation(zg[:, a:b2], p[0:DH, :], AF.Silu)

                # ---- depthwise causal conv + silu ----
                cv = tr.tile([DI, L], f32, tag="cv")
                cv2 = tr.tile([DI, L], f32, tag="cv2")
                nc.vector.tensor_scalar(cv[:], xcpad[:, 0:L], s_convW[:, 0:1],
                                        s_convB[:, 0:1], ALU.mult, ALU.add)
                nc.vector.scalar_tensor_tensor(cv2[:], xcpad[:, 1:1 + L],
                                               s_convW[:, 1:2], cv[:],
                                               ALU.mult, ALU.add)
                nc.vector.scalar_tensor_tensor(cv[:], xcpad[:, 2:2 + L],
                                               s_convW[:, 2:3], cv2[:],
                                               ALU.mult, ALU.add)
                nc.vector.scalar_tensor_tensor(cv2[:], xcpad[:, 3:3 + L],
                                               s_convW[:, 3:4], cv[:],
                                               ALU.mult, ALU.add)
                xc = blk.tile([DI, L], f32, tag="xc")
                nc.scalar.activation(xc[:], cv2[:], AF.Silu)
                xcb = blk.tile([DI, L], BF16, tag="xcb")
                nc.vector.tensor_copy(xcb[:], xc[:])

                # ---- x_proj: dt_rank rows + replicated B/C supertiles ----
                dtr = tr.tile([DR, L], f32, tag="dtr")
                for _, a, b2 in halves():
                    p = px.tile([DI, H], f32, tag="px")
                    nc.tensor.matmul(p[0:DR, :], s_xprDtr[:], xcb[:, a:b2],
                                     start=True, stop=True)
                    nc.scalar.activation(dtr[:, a:b2], p[0:DR, :], AF.Copy)
                bt = [rep.tile([128, L], BF16, tag=f"bt{j}", name=f"bt{j}")
                      for j in range(NJ)]
                ct = [rep.tile([128, L], BF16, tag=f"ct{j}", name=f"ct{j}")
                      for j in range(NJ)]
                for j in range(NJ):
                    for hh, a, b2 in halves():
                        p = px.tile([DI, H], f32, tag="px")
                        nc.tensor.matmul(p[:], s_xprB[:, j * 128:(j + 1) * 128],
                                         xcb[:, a:b2], start=True, stop=True)
                        if j % 2 == 0:
                            nc.scalar.activation(bt[j][:, a:b2], p[:], AF.Copy)
                        else:
                            nc.vector.tensor_copy(bt[j][:, a:b2], p[:])
                    for hh, a, b2 in halves():
                        p = px.tile([DI, H], f32, tag="px")
                        nc.tensor.matmul(p[:], s_xprC[:, j * 128:(j + 1) * 128],
                                         xcb[:, a:b2], start=True, stop=True)
                        if j % 2 == 0:
                            nc.vector.tensor_copy(ct[j][:, a:b2], p[:])
                        else:
                            nc.scalar.activation(ct[j][:, a:b2], p[:], AF.Copy)

                # ---- dt = softplus(dtr @ dtw.T + b) (positive) ----
                sdt = tr.tile([DI, L], f32, tag="sdt")
                for _, a, b2 in halves():
                    p = px.tile([DI, H], f32, tag="px")
                    nc.tensor.matmul(p[:], s_dtwT[:], dtr[:, a:b2],
                                     start=True, stop=True)
                    nc.scalar.activation(sdt[:, a:b2], p[:], AF.Sigmoid,
                                         scale=-1.0, bias=s_dtb[:, 0:1])
                dtn = tr.tile([DI, L], f32, tag="cv", name="dtn")
                nc.scalar.activation(dtn[:], sdt[:], AF.Ln)
                dthi = tr.tile([DH, L], BF16, tag="dthi")
                nc.vector.tensor_copy(dthi[:], dtn[0:DH, :])
                dtxn = tr.tile([DH, L], BF16, tag="dtxn")
                nc.vector.tensor_mul(dtxn[:], dtn[0:DH, :], xc[0:DH, :])

                # ---- selective scan over supertiles ----
                ys_ps = py.tile([DH, L], f32, tag="ys")
                first = True
                for rp in range(nrep):
                    for g in range(NG):
                        st_hi = stg.tile([8, L], BF16, tag="st_hi")
                        nc.sync.dma_start(st_hi[:], dthi[8 * g:8 * g + 8, :])
                        st_dx = stg.tile([8, L], BF16, tag="st_dx")
                        nc.sync.dma_start(st_dx[:], dtxn[8 * g:8 * g + 8, :])
                        pA = pa.tile([128, L], f32, tag="pA")
                        for _, a, b2 in halves():
                            nc.tensor.matmul(pA[:, a:b2], s_selB[:],
                                             st_hi[:, a:b2],
                                             start=True, stop=True)
                        dxb = lp.tile([128, L], BF16, tag="dxb", bufs=2)
                        for _, a, b2 in halves():
                            pX = px.tile([128, H], f32, tag="px")
                            nc.tensor.matmul(pX[:], s_selB[:], st_dx[:, a:b2],
                                             start=True, stop=True)
                            nc.scalar.activation(dxb[:, a:b2], pX[:], AF.Copy)
                        for j in range(NJ):
                            col = g * 16 + j
                            a_t = lp.tile([128, L], BF16, tag="a_t")
                            nc.scalar.activation(a_t[:], pA[:], AF.Exp,
                                                 scale=s_aposR[:, col:col + 1])
                            b_t = lp.tile([128, L], BF16, tag="b_t")
                            if j in POOL_B:
                                nc.gpsimd.tensor_mul(b_t[:], bt[j][:], dxb[:])
                            else:
                                nc.vector.tensor_mul(b_t[:], bt[j][:], dxb[:])
                            h_t = lp.tile([128, L], BF16, tag="h_t")
                            nc.vector.tensor_tensor_scan(
                                h_t[:], a_t[:], b_t[:], 0.0, ALU.mult, ALU.add)
                            m_t = lp.tile([128, L], BF16, tag="m_t")
                            if j in POOL_M:
                                nc.gpsimd.tensor_mul(m_t[:], h_t[:], ct[j][:])
                            else:
                                nc.vector.tensor_mul(m_t[:], h_t[:], ct[j][:])
                            last = (rp == nrep - 1 and g == NG - 1
                                    and j == NJ - 1)
                            for _, a, b2 in halves():
                                nc.tensor.matmul(
                                    ys_ps[:, a:b2],
                                    s_Z[:, 64 - 8 * g:128 - 8 * g],
                                    m_t[:, a:b2], start=first, stop=last,
                                    skip_group_check=True)
                            first = False
                ysf = pp.tile([DH, L], f32, tag="pp")
                nc.scalar.activation(ysf[:], ys_ps[:], AF.Copy)

                # ---- gate + out_proj (pre-scaled 0.5) ----
                g1 = pp.tile([DH, L], f32, tag="pp")
                nc.vector.scalar_tensor_tensor(g1[:], xc[0:DH, :],
                                               s_dcol[:, 0:1], ysf[:],
                                               ALU.mult, ALU.add)
                gated = gp.tile([DH, L], BF16, tag="gp")
                nc.vector.tensor_mul(gated[:], g1[:], zg[:])
                yo = gp.tile([128, L], BF16, tag="yob", bufs=1)
                for _, a, b2 in halves():
                    p = px.tile([DI, H], f32, tag="px")
                    nc.tensor.matmul(p[0:DM, :], s_outT[:], gated[:, a:b2],
                                     start=True, stop=True)
                    nc.scalar.activation(yo[0:DM, a:b2], p[0:DM, :], AF.Copy)
                yc = gp.tile([128, L], BF16, tag="ycb", bufs=1)
                nc.gpsimd.indirect_copy(yc[:], yo[:], s_irev[:], True)

                # ---- AllReduce over the 4-core batch group (dirs x halves),
                #      out_proj pre-scaled 0.5 -> (yf+yb)/2 ----
                cc_in = dram.tile([DM, L], BF16, tag="cci" + tag)
                cc_out = dram.tile([DM, L], BF16, tag="cco" + tag)
                nc.gpsimd.dma_start(cc_in[:], yc[0:DM, :])
                if no_cc:
                    nc.gpsimd.dma_start(cc_out[:], cc_in[:])
                else:
                    nc.gpsimd.collective_compute(
                        "AllReduce", ALU.add,
                        replica_groups=[[0, 1, 2, 3], [4, 5, 6, 7]],
                        ins=[cc_in.opt()], outs=[cc_out.opt()])
                ysum = gp.tile([DM, L], BF16, tag="ysb", bufs=1)
                nc.gpsimd.dma_start(ysum[:], cc_out[:])

                # ---- residual + layernorm (canonical order) ----
                rsd = pp.tile([DM, L], f32, tag="pp")
                nc.vector.tensor_add(rsd[:], base_canon[0:DM, :], ysum[:])
                mu = rw.tile([1, L], f32, tag="rw")
                sq = pp.tile([DM, L], f32, tag="pp")
                nc.vector.tensor_mul(sq[:], rsd[:], rsd[:])
                lnv = rw.tile([1, L], f32, tag="rw")
                cen = pp.tile([DM, L], f32, tag="pp")
                for _, a, b2 in halves():
                    p = px.tile([DI, H], f32, tag="px")
                    nc.tensor.matmul(p[0:1, :], onesdm_col[:], rsd[:, a:b2],
                                     start=True, stop=True)
                    nc.scalar.activation(mu[0:1, a:b2], p[0:1, :], AF.Copy,
                                         scale=1.0 / DM)
                for _, a, b2 in halves():
                    p = px.tile([DI, H], f32, tag="px")
                    nc.tensor.matmul(p[0:DM, :], onesdm_row[:], mu[0:1, a:b2],
                                     start=True, stop=True)
                    nc.vector.tensor_sub(cen[:, a:b2], rsd[:, a:b2],
                                         p[0:DM, :])
                for _, a, b2 in halves():
                    p = px.tile([DI, H], f32, tag="px")
                    nc.tensor.matmul(p[0:1, :], onesdm_col[:], sq[:, a:b2],
                                     start=True, stop=True)
                    nc.scalar.activation(lnv[0:1, a:b2], p[0:1, :], AF.Copy,
                                         scale=1.0 / DM)
                musq = rw.tile([1, L], f32, tag="rw")
                nc.vector.tensor_mul(musq[:], mu[:], mu[:])
                varv = rw.tile([1, L], f32, tag="rw")
                nc.vector.tensor_sub(varv[:], lnv[:], musq[:])
                lvar = rw.tile([1, L], f32, tag="rw")
                nc.scalar.activation(lvar[:], varv[:], AF.Ln,
                                     bias=eps_t[0:1, 0:1])
                rstd = rw.tile([1, L], f32, tag="rw")
                nc.scalar.activation(rstd[:], lvar[:], AF.Exp, scale=-0.5)
                nrm = pp.tile([DM, L], f32, tag="pp")
                for _, a, b2 in halves():
                    p = px.tile([DI, H], f32, tag="px")
                    nc.tensor.matmul(p[0:DM, :], onesdm_row[:],
                                     rstd[0:1, a:b2], start=True, stop=True)
                    nc.vector.tensor_mul(nrm[:, a:b2], cen[:, a:b2],
                                         p[0:DM, :])
                hn = hp.tile([128, L], f32, tag="hn")
                nc.vector.tensor_scalar(hn[0:DM, :], nrm[:], s_lng[:, 0:1],
                                        s_lnb[:, 0:1], ALU.mult, ALU.add)
                return hn

            # ---- block 1 ----
            h0 = res.tile([128, L], f32, tag="h0")
            for _, a, b2 in halves():
                p = px.tile([DI, H], f32, tag="px")
                nc.tensor.matmul(p[0:DM, :], s_fccT[:], s_embpos[:, a:b2],
                                 start=True, stop=True)
                nc.vector.tensor_scalar(h0[0:DM, a:b2], p[0:DM, :],
                                        s_fccb[:, 0:1], 0.0, ALU.add, ALU.max)
            h0cw = res.tile([128, L], f32, tag="h0cw")
            nc.gpsimd.indirect_copy(h0cw[:], h0[:], s_irev[:], True)

            h2 = mamba_block(h0, h0cw, "b1")

            # ---- transition: site-major -> cell-major ----
            h2t2 = res.tile([DM, L], f32, tag="h2t2")
            nc.vector.tensor_copy(
                h2t2[:].rearrange("p (c s) -> p c s", s=NSITE),
                h2[0:DM, :].rearrange("p (s c) -> p s c", c=NCELL)
                .transpose([0, 2, 1]))
            u2 = res.tile([128, L], f32, tag="u2")
            nc.gpsimd.indirect_copy(u2[:], h2[:], s_idx2[:], True)

            h3 = mamba_block(u2, h2t2, "b2")
            nc.sync.dma_start(out_h[:], h3[0:DM, :])

    return nc


# ---------------------------------------------------------------------------
# host side
# ---------------------------------------------------------------------------

def _pos_enc(D, Hh, W):
    pe = np.zeros((D, Hh, W), np.float32)
    dm = D // 2
    div = np.exp(np.arange(0, dm, 2, dtype=np.float32) * -(math.log(10000.0) / dm))
    pw = np.arange(W, dtype=np.float32)[:, None]
    ph = np.arange(Hh, dtype=np.float32)[:, None]
    pe[0:dm:2] = np.broadcast_to(np.sin(pw * div).T[:, None, :], (dm // 2, Hh, W))
    pe[1:dm:2] = np.broadcast_to(np.cos(pw * div).T[:, None, :], (dm // 2, Hh, W))
    pe[dm::2] = np.broadcast_to(np.sin(ph * div).T[:, :, None], (dm // 2, Hh, W))
    pe[dm + 1::2] = np.broadcast_to(np.cos(ph * div).T[:, :, None], (dm // 2, Hh, W))
    return pe.transpose(1, 2, 0)  # (H, W, D)


def _wrap_idx(vec):
    """indirect_copy index layout: index j lives at (partition j%16,
    slot j//16), replicated for each 16-partition group."""
    w = np.zeros((128, L // 16), np.uint16)
    blkv = vec.reshape(L // 16, 16).T.astype(np.uint16)
    for g in range(128 // 16):
        w[g * 16:(g + 1) * 16, :] = blkv
    return w


def make_in_maps(inputs):
    x = np.asarray(inputs["x"], np.float32)
    y = np.asarray(inputs["y"]).astype(np.int64)
    ci = np.asarray(inputs["cell_indices"]).astype(np.int64)
    cellEB = np.asarray(inputs["cellEB"], np.float32)
    CpGEB = np.asarray(inputs["CpGEB"], np.float32)
    fcc_w = np.asarray(inputs["fcc_w"], np.float32)
    fcc_b = np.asarray(inputs["fcc_b"], np.float32)
    ln_g = np.asarray(inputs["ln_g"], np.float32)
    ln_b = np.asarray(inputs["ln_b"], np.float32)
    in_proj_w = np.asarray(inputs["in_proj_w"], np.float32)
    conv_w = np.asarray(inputs["conv_w"], np.float32)
    conv_b = np.asarray(inputs["conv_b"], np.float32)
    x_proj_w = np.asarray(inputs["x_proj_w"], np.float32)
    dt_proj_w = np.asarray(inputs["dt_proj_w"], np.float32)
    dt_proj_b = np.asarray(inputs["dt_proj_b"], np.float32)
    A_log = np.asarray(inputs["A_log"], np.float32)
    D_param = np.asarray(inputs["D_param"], np.float32)
    out_proj_w = np.asarray(inputs["out_proj_w"], np.float32)

    pos = _pos_enc(3 * DIM, NSITE, NCELL)          # (site, cell, 96)
    pos_t1 = pos.reshape(L, 3 * DIM)

    emb = np.concatenate([
        CpGEB[y],                                   # (B, site, cell, 32)
        np.broadcast_to(cellEB[ci][:, None], (B, NSITE, NCELL, DIM)),
        np.broadcast_to(x[:, :, None, :], (B, NSITE, NCELL, DIM)),
    ], axis=-1).reshape(B, L, 3 * DIM)

    Apos = np.exp(A_log)                            # |A| = -A, (DI, DS)

    # supertile selectors (partition p = s_local*8 + d_local)
    p_ar = np.arange(128)
    selB = (p_ar[None, :] % 8 == np.arange(8)[:, None]).astype(bfloat16)
    Zsel = np.zeros((128, 128), bfloat16)
    Zsel[p_ar, 64 + p_ar % 8] = 1

    idx_id = np.arange(L, dtype=np.int64)
    idx_rv = idx_id[::-1].copy()
    v = np.arange(L)
    c_, s_ = v // NSITE, v % NSITE
    perm0 = s_ * NCELL + c_

    in_maps = []
    for core in range(N_CORES):
        b = core >> 2
        dirb = (core >> 1) & 1
        dh = core & 1
        pi = np.concatenate([np.arange(dh * DH, dh * DH + DH),
                             np.arange((1 - dh) * DH, (1 - dh) * DH + DH)])
        e = emb[b] if dirb == 0 else emb[b][::-1]
        p1 = pos_t1 if dirb == 0 else pos_t1[::-1]

        # replicated/permuted x_proj weights for the B/C supertiles
        xprojRepB = np.empty((DI, NJ * 128), np.float32)
        xprojRepC = np.empty((DI, NJ * 128), np.float32)
        s_l, d_l = p_ar // 8, p_ar % 8
        for j in range(NJ):
            st = j * 16 + s_l                       # global state per lane
            xprojRepB[:, j * 128 + p_ar] = -x_proj_w[DR + st][:, pi].T
            xprojRepC[:, j * 128 + p_ar] = x_proj_w[DR + DS + st][:, pi].T

        # -|A| per supertile lane/column (negated: dt is positive here)
        Apos_p = Apos[pi]
        aposR = np.zeros((128, 128), np.float32)
        for g in range(NG):
            for j in range(NJ):
                aposR[:, g * 16 + j] = Apos_p[g * 8 + d_l, j * 16 + s_l]

        m = {
            "embpos_T": np.ascontiguousarray((e + p1).T),
            "fccT": np.ascontiguousarray(fcc_w.T),
            "fccb": fcc_b.reshape(DM, 1),
            "inprojT": np.ascontiguousarray(
                np.concatenate([in_proj_w[0:DI][pi], in_proj_w[DI:2 * DI][pi]],
                               axis=0).T),
            "convW": np.ascontiguousarray(conv_w[pi, 0, :]),
            "convB": conv_b[pi].reshape(DI, 1),
            "xprojDtrT": np.ascontiguousarray(
                x_proj_w[0:DR][:, pi].T).astype(bfloat16),
            "xprojRepB": xprojRepB.astype(bfloat16),
            "xprojRepC": xprojRepC.astype(bfloat16),
            "dtwT": np.ascontiguousarray(dt_proj_w[pi].T),
            "dtb": (-dt_proj_b[pi]).reshape(DI, 1),
            "aposR": aposR,
            "dcol": D_param[pi[:DH]].reshape(DH, 1),
            "lng": ln_g.reshape(DM, 1),
            "lnb": ln_b.reshape(DM, 1),
            "outT": np.ascontiguousarray(
                out_proj_w[:, pi[:DH]].T).astype(bfloat16) * bfloat16(0.5),
            "selB": selB,
            "Zsel": Zsel,
            "idx_rev": _wrap_idx(idx_id if dirb == 0 else idx_rv),
            "idx2": _wrap_idx(perm0 if dirb == 0 else perm0[::-1]),
        }
        in_maps.append(m)
    return in_maps


def postprocess(results):
    out = np.zeros((B, NSITE, NCELL, DM), np.float32)
    for b, core in ((0, 0), (1, 4)):
        h3 = results[core]["out"]                   # (DM, L) t2-canonical
        seq = h3.T.reshape(NCELL, NSITE, DM)        # v = c*NSITE + s
        out[b] = seq.transpose(1, 0, 2)
    return out


# ---------------------------------------------------------------------------
# cached PJRT runner (built once per process; repeat kernel() calls are fast)
# ---------------------------------------------------------------------------
import time

import jax
from jax.sharding import Mesh, PartitionSpec
from jax.experimental.shard_map import shard_map

from concourse.bass2jax import _bass_exec_p, install_neuronx_cc_hook, partition_id_tensor


class Runner:
    def __init__(self, nc, in_maps, n_cores=8):
        install_neuronx_cc_hook()
        self.n_cores = n_cores
        partition_name = nc.partition_id_tensor.name if nc.partition_id_tensor else None
        in_names, out_names, out_avals, zero_outs = [], [], [], []
        for alloc in nc.m.functions[0].allocations:
            if not isinstance(alloc, mybir.MemoryLocationSet):
                continue
            name = alloc.memorylocations[0].name
            if alloc.kind == "ExternalInput":
                if name != partition_name:
                    in_names.append(name)
            elif alloc.kind == "ExternalOutput":
                out_names.append(name)
                shape = tuple(alloc.tensor_shape)
                dtype = mybir.dt.np(alloc.dtype)
                out_avals.append(jax.core.ShapedArray(shape, dtype))
                zero_outs.append(np.zeros(shape, dtype))
        n_params = len(in_names)
        n_outs = len(out_avals)
        all_in_names = list(in_names) + out_names
        if partition_name is not None:
            all_in_names.append(partition_name)
        donate = tuple(range(n_params, n_params + n_outs))

        def _body(*args):
            operands = list(args)
            if partition_name is not None:
                operands.append(partition_id_tensor())
            outs = _bass_exec_p.bind(
                *operands,
                out_avals=tuple(out_avals),
                in_names=tuple(all_in_names),
                out_names=tuple(out_names),
                lowering_input_output_aliases=(),
                sim_require_finite=True,
                sim_require_nnan=True,
                nc=nc,
            )
            return tuple(outs)

        devices = jax.devices()[:n_cores]
        mesh = Mesh(np.asarray(devices), ("core",))
        in_specs = (PartitionSpec("core"),) * (n_params + n_outs)
        out_specs = (PartitionSpec("core"),) * n_outs
        self.f = jax.jit(
            shard_map(_body, mesh=mesh, in_specs=in_specs,
                      out_specs=out_specs, check_rep=False),
            donate_argnums=donate, keep_unused=True)
        self.in_names = in_names
        self.n_params = n_params
        self.sharding = jax.sharding.NamedSharding(mesh, PartitionSpec("core"))
        self.set_inputs(in_maps)
        zshapes = [(n_cores * z.shape[0], *z.shape[1:]) for z in zero_outs]
        zdt = [z.dtype for z in zero_outs]

        def _mkzeros():
            return tuple(jax.numpy.zeros(s, d) for s, d in zip(zshapes, zdt))

        self.mkzeros = jax.jit(_mkzeros, out_shardings=(self.sharding,) * n_outs)
        self.out_names = out_names
        self.out_avals = out_avals

    def set_inputs(self, in_maps):
        per_core = [[np.asarray(m[n]) for n in self.in_names] for m in in_maps]
        concat_in = [
            np.concatenate([per_core[c][i] for c in range(self.n_cores)], axis=0)
            for i in range(self.n_params)
        ]
        self.inputs_dev = [jax.device_put(a, self.sharding) for a in concat_in]

    def run(self):
        z = self.mkzeros()
        jax.block_until_ready(z)
        t0 = time.time()
        outs = self.f(*self.inputs_dev, *z)
        jax.block_until_ready(outs)
        dt = time.time() - t0
        return outs, dt

    def results(self, outs):
        res = []
        for c in range(self.n_cores):
            m = {}
            for i, name in enumerate(self.out_names):
                a = np.asarray(outs[i])
                m[name] = a.reshape(self.n_cores, *self.out_avals[i].shape)[c]
            res.append(m)
        return res

    def bench(self, warmup=2, iters=12):
        for _ in range(warmup):
            self.run()
        ts = []
        for _ in range(iters):
            _, dt = self.run()
            ts.append(dt)
        ts.sort()
        return ts[len(ts) // 2], ts[0]


_cache = {}


def _get_nc(nrep=1):
    if nrep not in _cache:
        _cache[nrep] = build_bass(nrep)
    return _cache[nrep]


_runner_cache = {}


def get_runner(inputs, nrep=1):
    key = nrep
    if key not in _runner_cache:
        _runner_cache[key] = Runner(_get_nc(nrep), make_in_maps(inputs), N_CORES)
    return _runner_cache[key]


def kernel(**inputs) -> np.ndarray:
    r = get_runner(inputs, 1)
    # refresh device inputs in case the caller passes different data
    in_maps = make_in_maps(inputs)
    r.set_inputs(in_maps)
    outs, _ = r.run()
    return postprocess(r.results(outs))


# revision 12
# speedup vs baseline: 1.0927x; 1.0053x over previous
"""Bidirectional Mamba block (nn_BiMamba) on 8 Trainium2 NeuronCores.

Sharding: core c = b*4 + dir*2 + dh -> (batch b, direction dir, d_inner-half
dh).  Host permutes the d_inner axis per core so the core's own 64 channels
are rows 0:64 of every phase tensor.  The selective scan runs on
"supertiles" [128, L]: partition p = 16-state-group x 8-channel-group
(p = s_local*8 + d_local), so one PE broadcast + one ACT exp serves 8
channels at once.  B/C are materialized replicated (x_proj with
host-replicated lhsT columns) so the scan-input multiply runs at DVE 2x.
Each core owns 64 channels x all 256 states -> the state reduction is
core-local and only ONE AllReduce (over the 4-core batch group: 2 dirs x 2
halves) is needed per mamba layer, with out_proj pre-scaled by 0.5.
Direction flips / layer reorders use host-fed gather indices (SPMD).
"""

import json
import math

import numpy as np
from ml_dtypes import bfloat16

import concourse.bass as bass
import concourse.mybir as mybir
import concourse.tile as tile

AF = mybir.ActivationFunctionType
ALU = mybir.AluOpType
F32 = mybir.dt.float32
BF16 = mybir.dt.bfloat16

# problem dims (hardcoded per task contract)
B, NSITE, NCELL, DIM = 2, 128, 8, 32
DM, DI, DS, DR, DCONV = 2 * DIM, 4 * DIM, 256, 4, 4
L = NSITE * NCELL            # 1024
DH = DI // 2                 # 64 channels per core
NG = DH // 8                 # 8 channel-groups of 8
NJ = DS // 16                # 16 state-groups of 16
N_CORES = 8
EPS = 1e-5
H = 512                      # matmul free-dim chunk (PSUM bank)

# per-(g,j): which engine does the b-multiply / m-multiply.
# 'P' = Pool, 'V' = DVE.  Tuned for engine balance (DVE also runs scans).
POOL_B = {0, 3, 6, 7, 9, 12, 15}       # bmul on Pool for these j
POOL_M = {1, 4, 5, 11, 13, 14}         # mmul on Pool for these j
# -> per g: Pool 13 tiles, DVE 16 scans + 9 bmul + 10 mmul


# ---------------------------------------------------------------------------
# BIR post-processing: this walrus build accepts at most ONE sync wait per
# instruction; hoist excess waits onto standalone NoOp carriers.
# ---------------------------------------------------------------------------

def _split_waits(bir_json: bytes, maxw: int = 1) -> bytes:
    data = json.loads(bir_json)
    n = [0]

    def carrier(ins, waits):
        n[0] += 1
        return {
            "debug": ins.get("debug", 0),
            "engine": ins["engine"],
            "ins": [],
            "name": f"I-wsplit-{n[0]}",
            "opcode": "NoOp",
            "outs": [],
            "sync_info": {"on_update": [], "on_wait": waits},
        }

    for fn in data["functions"]:
        for blk in fn["blocks"]:
            out = []
            for ins in blk["instructions"]:
                si = ins.get("sync_info")
                if si and si.get("on_wait") and len(si["on_wait"]) > maxw:
                    waits = si["on_wait"]
                    extra, keep = waits[:-maxw], waits[-maxw:]
                    for i in range(0, len(extra), maxw):
                        out.append(carrier(ins, extra[i:i + maxw]))
                    si["on_wait"] = keep
                out.append(ins)
            blk["instructions"] = out
    return json.dumps(data).encode()


_orig_to_json_bytes = bass.Bass.to_json_bytes


def _patched_to_json_bytes(self, *a, **k):
    return _split_waits(_orig_to_json_bytes(self, *a, **k))


bass.Bass.to_json_bytes = _patched_to_json_bytes

# ---------------------------------------------------------------------------
# device program
# ---------------------------------------------------------------------------

def build_bass(nrep: int = 1, no_cc: bool = False):
    nc = bass.Bass(num_devices=N_CORES)
    f32 = F32

    def din(name, shape, dtype=f32):
        return nc.dram_tensor(name, shape, dtype, kind="ExternalInput")

    embpos_T = din("embpos_T", [3 * DIM, L])
    fccT = din("fccT", [3 * DIM, DM])
    fccb = din("fccb", [DM, 1])
    inprojT = din("inprojT", [DM, 2 * DI])
    convW = din("convW", [DI, DCONV])
    convB = din("convB", [DI, 1])
    xprojDtrT = din("xprojDtrT", [DI, DR], BF16)
    xprojRepB = din("xprojRepB", [DI, NJ * 128], BF16)
    xprojRepC = din("xprojRepC", [DI, NJ * 128], BF16)
    dtwT = din("dtwT", [DR, DI])
    dtb = din("dtb", [DI, 1])            # -dt_proj_b
    aposR = din("aposR", [128, 128])       # col g*16+j: -|A| for (s,d) lanes
    dcol = din("dcol", [DH, 1])
    lng = din("lng", [DM, 1])
    lnb = din("lnb", [DM, 1])
    outT = din("outT", [DH, DM], BF16)     # pre-scaled by 0.5
    selB = din("selB", [8, 128], BF16)     # d-broadcast selector
    Zsel = din("Zsel", [128, 128], BF16)   # reduce row-placement selector
    idx_rev = din("idx_rev", [128, L // 16], mybir.dt.uint16)
    idx2 = din("idx2", [128, L // 16], mybir.dt.uint16)

    out_h = nc.dram_tensor("out", [DM, L], f32, kind="ExternalOutput")

    with tile.TileContext(nc) as tc:
        with (
            tc.tile_pool(name="cst", bufs=1) as cst,
            tc.tile_pool(name="res", bufs=1) as res,
            tc.tile_pool(name="blk", bufs=1) as blk,
            tc.tile_pool(name="rep", bufs=1) as rep,
            tc.tile_pool(name="tr", bufs=1) as tr,
            tc.tile_pool(name="pp", bufs=3) as pp,
            tc.tile_pool(name="hp", bufs=2) as hp,
            tc.tile_pool(name="gp", bufs=2) as gp,
            tc.tile_pool(name="rw", bufs=3) as rw,
            tc.tile_pool(name="stg", bufs=2) as stg,
            tc.tile_pool(name="lp", bufs=3) as lp,
            tc.tile_pool(name="pa", bufs=2, space="PSUM") as pa,
            tc.tile_pool(name="px", bufs=2, space="PSUM") as px,
            tc.tile_pool(name="py", bufs=1, space="PSUM") as py,
            tc.tile_pool(name="dram", bufs=2, space="DRAM") as dram,
        ):
            def load(t, shape, dtype=f32):
                s = cst.tile(shape, dtype, tag=t.name)
                nc.sync.dma_start(s[:], t[:])
                return s

            s_embpos = load(embpos_T, [3 * DIM, L])
            s_fccT = load(fccT, [3 * DIM, DM])
            s_fccb = load(fccb, [DM, 1])
            s_inpT = load(inprojT, [DM, 2 * DI])
            s_convW = load(convW, [DI, DCONV])
            s_convB = load(convB, [DI, 1])
            s_xprDtr = load(xprojDtrT, [DI, DR], BF16)
            s_xprB = load(xprojRepB, [DI, NJ * 128], BF16)
            s_xprC = load(xprojRepC, [DI, NJ * 128], BF16)
            s_dtwT = load(dtwT, [DR, DI])
            s_dtb = load(dtb, [DI, 1])
            s_aposR = load(aposR, [128, 128])
            s_dcol = load(dcol, [DH, 1])
            s_lng = load(lng, [DM, 1])
            s_lnb = load(lnb, [DM, 1])
            s_outT = load(outT, [DH, DM], BF16)
            s_selB = load(selB, [8, 128], BF16)
            s_Z = load(Zsel, [128, 128], BF16)
            s_irev = load(idx_rev, [128, L // 16], mybir.dt.uint16)
            s_idx2 = load(idx2, [128, L // 16], mybir.dt.uint16)

            onesdm_col = cst.tile([DM, 1], f32, tag="onesdm_col")
            nc.vector.memset(onesdm_col[:], 1.0)
            onesdm_row = cst.tile([1, DM], f32, tag="onesdm_row")
            nc.vector.memset(onesdm_row[:], 1.0)
            eps_t = cst.tile([1, 1], f32, tag="eps_t")
            nc.vector.memset(eps_t[:], EPS)

            def halves():
                return [(h0, h0 * H, (h0 + 1) * H) for h0 in range(L // H)]

            def mamba_block(u, base_canon, tag):
                """u: (128, L) f32 tile, rows 0:DM valid, local-order input.
                base_canon: residual base in canonical order (rows 0:DM).
                Returns h_next (128, L) canonical order (rows 0:DM)."""
                # ---- in_proj ----
                xcpad = blk.tile([DI, DCONV - 1 + L], f32, tag="xcpad")
                nc.vector.memset(xcpad[:, 0:DCONV - 1], 0.0)
                zg = blk.tile([DH, L], f32, tag="zg")
                for _, a, b2 in halves():
                    p = px.tile([DI, H], f32, tag="px")
                    nc.tensor.matmul(p[:], s_inpT[:, 0:DI], u[0:DM, a:b2],
                                     start=True, stop=True)
                    nc.scalar.activation(xcpad[:, DCONV - 1 + a:DCONV - 1 + b2],
                                         p[:], AF.Copy)
                for _, a, b2 in halves():
                    p = px.tile([DI, H], f32, tag="px")
                    nc.tensor.matmul(p[0:DH, :], s_inpT[:, DI:DI + DH],
                                     u[0:DM, a:b2], start=True, stop=True)
                    nc.scalar.activation(zg[:, a:b2], p[0:DH, :], AF.Silu)

                # ---- depthwise causal conv + silu ----
                cv = tr.tile([DI, L], f32, tag="cv")
                cv2 = tr.tile([DI, L], f32, tag="cv2")
                nc.vector.tensor_scalar(cv[:], xcpad[:, 0:L], s_convW[:, 0:1],
                                        s_convB[:, 0:1], ALU.mult, ALU.add)
                nc.vector.scalar_tensor_tensor(cv2[:], xcpad[:, 1:1 + L],
                                               s_convW[:, 1:2], cv[:],
                                               ALU.mult, ALU.add)
                nc.vector.scalar_tensor_tensor(cv[:], xcpad[:, 2:2 + L],
                                               s_convW[:, 2:3], cv2[:],
                                               ALU.mult, ALU.add)
                nc.vector.scalar_tensor_tensor(cv2[:], xcpad[:, 3:3 + L],
                                               s_convW[:, 3:4], cv[:],
                                               ALU.mult, ALU.add)
                xc = blk.tile([DI, L], f32, tag="xc")
                nc.scalar.activation(xc[:], cv2[:], AF.Silu)
                xcb = blk.tile([DI, L], BF16, tag="xcb")
                nc.vector.tensor_copy(xcb[:], xc[:])

                # ---- x_proj: dt_rank rows + replicated B/C supertiles ----
                dtr = tr.tile([DR, L], f32, tag="dtr")
                for _, a, b2 in halves():
                    p = px.tile([DI, H], f32, tag="px")
                    nc.tensor.matmul(p[0:DR, :], s_xprDtr[:], xcb[:, a:b2],
                                     start=True, stop=True)
                    nc.scalar.activation(dtr[:, a:b2], p[0:DR, :], AF.Copy)
                bt = [rep.tile([128, L], BF16, tag=f"bt{j}", name=f"bt{j}")
                      for j in range(NJ)]
                ct = [rep.tile([128, L], BF16, tag=f"ct{j}", name=f"ct{j}")
                      for j in range(NJ)]
                for j in range(NJ):
                    for hh, a, b2 in halves():
                        p = px.tile([DI, H], f32, tag="px")
                        nc.tensor.matmul(p[:], s_xprB[:, j * 128:(j + 1) * 128],
                                         xcb[:, a:b2], start=True, stop=True)
                        if j % 2 == 0:
                            nc.scalar.activation(bt[j][:, a:b2], p[:], AF.Copy)
                        else:
                            nc.vector.tensor_copy(bt[j][:, a:b2], p[:])
                    for hh, a, b2 in halves():
                        p = px.tile([DI, H], f32, tag="px")
                        nc.tensor.matmul(p[:], s_xprC[:, j * 128:(j + 1) * 128],
                                         xcb[:, a:b2], start=True, stop=True)
                        if j % 2 == 0:
                            nc.vector.tensor_copy(ct[j][:, a:b2], p[:])
                        else:
                            nc.scalar.activation(ct[j][:, a:b2], p[:], AF.Copy)

                # ---- dt = softplus(dtr @ dtw.T + b) (positive) ----
                sdt = tr.tile([DI, L], f32, tag="sdt")
                for _, a, b2 in halves():
                    p = px.tile([DI, H], f32, tag="px")
                    nc.tensor.matmul(p[:], s_dtwT[:], dtr[:, a:b2],
                                     start=True, stop=True)
                    nc.scalar.activation(sdt[:, a:b2], p[:], AF.Sigmoid,
                                         scale=-1.0, bias=s_dtb[:, 0:1])
                dtn = tr.tile([DI, L], f32, tag="cv", name="dtn")
                nc.scalar.activation(dtn[:], sdt[:], AF.Ln)
                dthi = tr.tile([DH, L], BF16, tag="dthi")
                nc.vector.tensor_copy(dthi[:], dtn[0:DH, :])
                dtxn = tr.tile([DH, L], BF16, tag="dtxn")
                nc.vector.tensor_mul(dtxn[:], dtn[0:DH, :], xc[0:DH, :])

                # ---- selective scan over supertiles ----
                ys_ps = py.tile([DH, L], f32, tag="ys")
                first = True
                for rp in range(nrep):
                    for g in range(NG):
                        st_hi = stg.tile([8, L], BF16, tag="st_hi")
                        nc.sync.dma_start(st_hi[:], dthi[8 * g:8 * g + 8, :])
                        st_dx = stg.tile([8, L], BF16, tag="st_dx")
                        nc.sync.dma_start(st_dx[:], dtxn[8 * g:8 * g + 8, :])
                        pA = pa.tile([128, L], f32, tag="pA")
                        for _, a, b2 in halves():
                            nc.tensor.matmul(pA[:, a:b2], s_selB[:],
                                             st_hi[:, a:b2],
                                             start=True, stop=True)
                        dxb = lp.tile([128, L], BF16, tag="dxb", bufs=2)
                        for _, a, b2 in halves():
                            pX = px.tile([128, H], f32, tag="px")
                            nc.tensor.matmul(pX[:], s_selB[:], st_dx[:, a:b2],
                                             start=True, stop=True)
                            nc.scalar.activation(dxb[:, a:b2], pX[:], AF.Copy)
                        for j in range(NJ):
                            col = g * 16 + j
                            a_t = lp.tile([128, L], BF16, tag="a_t")
                            nc.scalar.activation(a_t[:], pA[:], AF.Exp,
                                                 scale=s_aposR[:, col:col + 1])
                            b_t = lp.tile([128, L], BF16, tag="b_t")
                            if j in POOL_B:
                                nc.gpsimd.tensor_mul(b_t[:], bt[j][:], dxb[:])
                            else:
                                for _, a, b2 in halves():
                                    nc.vector.tensor_mul(b_t[:, a:b2],
                                                         bt[j][:, a:b2],
                                                         dxb[:, a:b2])
                            h_t = lp.tile([128, L], BF16, tag="h_t")
                            for c4 in range(4):
                                cs, ce = c4 * 256, (c4 + 1) * 256
                                nc.vector.tensor_tensor_scan(
                                    h_t[:, cs:ce], a_t[:, cs:ce],
                                    b_t[:, cs:ce],
                                    0.0 if c4 == 0 else h_t[:, cs - 1:cs],
                                    ALU.mult, ALU.add)
                            m_t = lp.tile([128, L], BF16, tag="m_t")
                            if j in POOL_M:
                                nc.gpsimd.tensor_mul(m_t[:], h_t[:], ct[j][:])
                            else:
                                for _, a, b2 in halves():
                                    nc.vector.tensor_mul(m_t[:, a:b2],
                                                         h_t[:, a:b2],
                                                         ct[j][:, a:b2])
                            last = (rp == nrep - 1 and g == NG - 1
                                    and j == NJ - 1)
                            for _, a, b2 in halves():
                                nc.tensor.matmul(
                                    ys_ps[:, a:b2],
                                    s_Z[:, 64 - 8 * g:128 - 8 * g],
                                    m_t[:, a:b2], start=first, stop=last,
                                    skip_group_check=True)
                            first = False
                ysf = pp.tile([DH, L], f32, tag="pp")
                nc.scalar.activation(ysf[:], ys_ps[:], AF.Copy)

                # ---- gate + out_proj (pre-scaled 0.5) ----
                g1 = pp.tile([DH, L], f32, tag="pp")
                nc.vector.scalar_tensor_tensor(g1[:], xc[0:DH, :],
                                               s_dcol[:, 0:1], ysf[:],
                                               ALU.mult, ALU.add)
                gated = gp.tile([DH, L], BF16, tag="gp")
                nc.vector.tensor_mul(gated[:], g1[:], zg[:])
                yo = gp.tile([128, L], BF16, tag="yob", bufs=1)
                for _, a, b2 in halves():
                    p = px.tile([DI, H], f32, tag="px")
                    nc.tensor.matmul(p[0:DM, :], s_outT[:], gated[:, a:b2],
                                     start=True, stop=True)
                    nc.scalar.activation(yo[0:DM, a:b2], p[0:DM, :], AF.Copy)
                yc = gp.tile([128, L], BF16, tag="ycb", bufs=1)
                nc.gpsimd.indirect_copy(yc[:], yo[:], s_irev[:], True)

                # ---- AllReduce over the 4-core batch group (dirs x halves),
                #      out_proj pre-scaled 0.5 -> (yf+yb)/2 ----
                cc_in = dram.tile([DM, L], BF16, tag="cci" + tag)
                cc_out = dram.tile([DM, L], BF16, tag="cco" + tag)
                nc.gpsimd.dma_start(cc_in[:], yc[0:DM, :])
                if no_cc:
                    nc.gpsimd.dma_start(cc_out[:], cc_in[:])
                else:
                    nc.gpsimd.collective_compute(
                        "AllReduce", ALU.add,
                        replica_groups=[[0, 1, 2, 3], [4, 5, 6, 7]],
                        ins=[cc_in.opt()], outs=[cc_out.opt()])
                ysum = gp.tile([DM, L], BF16, tag="ysb", bufs=1)
                nc.gpsimd.dma_start(ysum[:], cc_out[:])

                # ---- residual + layernorm (canonical order) ----
                rsd = pp.tile([DM, L], f32, tag="pp")
                nc.vector.tensor_add(rsd[:], base_canon[0:DM, :], ysum[:])
                mu = rw.tile([1, L], f32, tag="rw")
                sq = pp.tile([DM, L], f32, tag="pp")
                nc.vector.tensor_mul(sq[:], rsd[:], rsd[:])
                lnv = rw.tile([1, L], f32, tag="rw")
                cen = pp.tile([DM, L], f32, tag="pp")
                for _, a, b2 in halves():
                    p = px.tile([DI, H], f32, tag="px")
                    nc.tensor.matmul(p[0:1, :], onesdm_col[:], rsd[:, a:b2],
                                     start=True, stop=True)
                    nc.scalar.activation(mu[0:1, a:b2], p[0:1, :], AF.Copy,
                                         scale=1.0 / DM)
                for _, a, b2 in halves():
                    p = px.tile([DI, H], f32, tag="px")
                    nc.tensor.matmul(p[0:DM, :], onesdm_row[:], mu[0:1, a:b2],
                                     start=True, stop=True)
                    nc.vector.tensor_sub(cen[:, a:b2], rsd[:, a:b2],
                                         p[0:DM, :])
                for _, a, b2 in halves():
                    p = px.tile([DI, H], f32, tag="px")
                    nc.tensor.matmul(p[0:1, :], onesdm_col[:], sq[:, a:b2],
                                     start=True, stop=True)
                    nc.scalar.activation(lnv[0:1, a:b2], p[0:1, :], AF.Copy,
                                         scale=1.0 / DM)
                musq = rw.tile([1, L], f32, tag="rw")
                nc.vector.tensor_mul(musq[:], mu[:], mu[:])
                varv = rw.tile([1, L], f32, tag="rw")
                nc.vector.tensor_sub(varv[:], lnv[:], musq[:])
                lvar = rw.tile([1, L], f32, tag="rw")
                nc.scalar.activation(lvar[:], varv[:], AF.Ln,
                                     bias=eps_t[0:1, 0:1])
                rstd = rw.tile([1, L], f32, tag="rw")
                nc.scalar.activation(rstd[:], lvar[:], AF.Exp, scale=-0.5)
                nrm = pp.tile([DM, L], f32, tag="pp")
                for _, a, b2 in halves():
                    p = px.tile([DI, H], f32, tag="px")
                    nc.tensor.matmul(p[0:DM, :], onesdm_row[:],
                                     rstd[0:1, a:b2], start=True, stop=True)
                    nc.vector.tensor_mul(nrm[:, a:b2], cen[:, a:b2],
                                         p[0:DM, :])
                hn = hp.tile([128, L], f32, tag="hn")
                nc.vector.tensor_scalar(hn[0:DM, :], nrm[:], s_lng[:, 0:1],
                                        s_lnb[:, 0:1], ALU.mult, ALU.add)
                return hn

            # ---- block 1 ----
            h0 = res.tile([128, L], f32, tag="h0")
            for _, a, b2 in halves():
                p = px.tile([DI, H], f32, tag="px")
                nc.tensor.matmul(p[0:DM, :], s_fccT[:], s_embpos[:, a:b2],
                                 start=True, stop=True)
                nc.vector.tensor_scalar(h0[0:DM, a:b2], p[0:DM, :],
                                        s_fccb[:, 0:1], 0.0, ALU.add, ALU.max)
            h0cw = res.tile([128, L], f32, tag="h0cw")
            nc.gpsimd.indirect_copy(h0cw[:], h0[:], s_irev[:], True)

            h2 = mamba_block(h0, h0cw, "b1")

            # ---- transition: site-major -> cell-major ----
            h2t2 = res.tile([DM, L], f32, tag="h2t2")
            nc.vector.tensor_copy(
                h2t2[:].rearrange("p (c s) -> p c s", s=NSITE),
                h2[0:DM, :].rearrange("p (s c) -> p s c", c=NCELL)
                .transpose([0, 2, 1]))
            u2 = res.tile([128, L], f32, tag="u2")
            nc.gpsimd.indirect_copy(u2[:], h2[:], s_idx2[:], True)

            h3 = mamba_block(u2, h2t2, "b2")
            nc.sync.dma_start(out_h[:], h3[0:DM, :])

    return nc


# ---------------------------------------------------------------------------
# host side
# ---------------------------------------------------------------------------

def _pos_enc(D, Hh, W):
    pe = np.zeros((D, Hh, W), np.float32)
    dm = D // 2
    div = np.exp(np.arange(0, dm, 2, dtype=np.float32) * -(math.log(10000.0) / dm))
    pw = np.arange(W, dtype=np.float32)[:, None]
    ph = np.arange(Hh, dtype=np.float32)[:, None]
    pe[0:dm:2] = np.broadcast_to(np.sin(pw * div).T[:, None, :], (dm // 2, Hh, W))
    pe[1:dm:2] = np.broadcast_to(np.cos(pw * div).T[:, None, :], (dm // 2, Hh, W))
    pe[dm::2] = np.broadcast_to(np.sin(ph * div).T[:, :, None], (dm // 2, Hh, W))
    pe[dm + 1::2] = np.broadcast_to(np.cos(ph * div).T[:, :, None], (dm // 2, Hh, W))
    return pe.transpose(1, 2, 0)  # (H, W, D)


def _wrap_idx(vec):
    """indirect_copy index layout: index j lives at (partition j%16,
    slot j//16), replicated for each 16-partition group."""
    w = np.zeros((128, L // 16), np.uint16)
    blkv = vec.reshape(L // 16, 16).T.astype(np.uint16)
    for g in range(128 // 16):
        w[g * 16:(g + 1) * 16, :] = blkv
    return w


def make_in_maps(inputs):
    x = np.asarray(inputs["x"], np.float32)
    y = np.asarray(inputs["y"]).astype(np.int64)
    ci = np.asarray(inputs["cell_indices"]).astype(np.int64)
    cellEB = np.asarray(inputs["cellEB"], np.float32)
    CpGEB = np.asarray(inputs["CpGEB"], np.float32)
    fcc_w = np.asarray(inputs["fcc_w"], np.float32)
    fcc_b = np.asarray(inputs["fcc_b"], np.float32)
    ln_g = np.asarray(inputs["ln_g"], np.float32)
    ln_b = np.asarray(inputs["ln_b"], np.float32)
    in_proj_w = np.asarray(inputs["in_proj_w"], np.float32)
    conv_w = np.asarray(inputs["conv_w"], np.float32)
    conv_b = np.asarray(inputs["conv_b"], np.float32)
    x_proj_w = np.asarray(inputs["x_proj_w"], np.float32)
    dt_proj_w = np.asarray(inputs["dt_proj_w"], np.float32)
    dt_proj_b = np.asarray(inputs["dt_proj_b"], np.float32)
    A_log = np.asarray(inputs["A_log"], np.float32)
    D_param = np.asarray(inputs["D_param"], np.float32)
    out_proj_w = np.asarray(inputs["out_proj_w"], np.float32)

    pos = _pos_enc(3 * DIM, NSITE, NCELL)          # (site, cell, 96)
    pos_t1 = pos.reshape(L, 3 * DIM)

    emb = np.concatenate([
        CpGEB[y],                                   # (B, site, cell, 32)
        np.broadcast_to(cellEB[ci][:, None], (B, NSITE, NCELL, DIM)),
        np.broadcast_to(x[:, :, None, :], (B, NSITE, NCELL, DIM)),
    ], axis=-1).reshape(B, L, 3 * DIM)

    Apos = np.exp(A_log)                            # |A| = -A, (DI, DS)

    # supertile selectors (partition p = s_local*8 + d_local)
    p_ar = np.arange(128)
    selB = (p_ar[None, :] % 8 == np.arange(8)[:, None]).astype(bfloat16)
    Zsel = np.zeros((128, 128), bfloat16)
    Zsel[p_ar, 64 + p_ar % 8] = 1

    idx_id = np.arange(L, dtype=np.int64)
    idx_rv = idx_id[::-1].copy()
    v = np.arange(L)
    c_, s_ = v // NSITE, v % NSITE
    perm0 = s_ * NCELL + c_

    in_maps = []
    for core in range(N_CORES):
        b = core >> 2
        dirb = (core >> 1) & 1
        dh = core & 1
        pi = np.concatenate([np.arange(dh * DH, dh * DH + DH),
                             np.arange((1 - dh) * DH, (1 - dh) * DH + DH)])
        e = emb[b] if dirb == 0 else emb[b][::-1]
        p1 = pos_t1 if dirb == 0 else pos_t1[::-1]

        # replicated/permuted x_proj weights for the B/C supertiles
        xprojRepB = np.empty((DI, NJ * 128), np.float32)
        xprojRepC = np.empty((DI, NJ * 128), np.float32)
        s_l, d_l = p_ar // 8, p_ar % 8
        for j in range(NJ):
            st = j * 16 + s_l                       # global state per lane
            xprojRepB[:, j * 128 + p_ar] = -x_proj_w[DR + st][:, pi].T
            xprojRepC[:, j * 128 + p_ar] = x_proj_w[DR + DS + st][:, pi].T

        # -|A| per supertile lane/column (negated: dt is positive here)
        Apos_p = Apos[pi]
        aposR = np.zeros((128, 128), np.float32)
        for g in range(NG):
            for j in range(NJ):
                aposR[:, g * 16 + j] = Apos_p[g * 8 + d_l, j * 16 + s_l]

        m = {
            "embpos_T": np.ascontiguousarray((e + p1).T),
            "fccT": np.ascontiguousarray(fcc_w.T),
            "fccb": fcc_b.reshape(DM, 1),
            "inprojT": np.ascontiguousarray(
                np.concatenate([in_proj_w[0:DI][pi], in_proj_w[DI:2 * DI][pi]],
                               axis=0).T),
            "convW": np.ascontiguousarray(conv_w[pi, 0, :]),
            "convB": conv_b[pi].reshape(DI, 1),
            "xprojDtrT": np.ascontiguousarray(
                x_proj_w[0:DR][:, pi].T).astype(bfloat16),
            "xprojRepB": xprojRepB.astype(bfloat16),
            "xprojRepC": xprojRepC.astype(bfloat16),
            "dtwT": np.ascontiguousarray(dt_proj_w[pi].T),
            "dtb": (-dt_proj_b[pi]).reshape(DI, 1),
            "aposR": aposR,
            "dcol": D_param[pi[:DH]].reshape(DH, 1),
            "lng": ln_g.reshape(DM, 1),
            "lnb": ln_b.reshape(DM, 1),
            "outT": np.ascontiguousarray(
                out_proj_w[:, pi[:DH]].T).astype(bfloat16) * bfloat16(0.5),
            "selB": selB,
            "Zsel": Zsel,
            "idx_rev": _wrap_idx(idx_id if dirb == 0 else idx_rv),
            "idx2": _wrap_idx(perm0 if dirb == 0 else perm0[::-1]),
        }
        in_maps.append(m)
    return in_maps


def postprocess(results):
    out = np.zeros((B, NSITE, NCELL, DM), np.float32)
    for b, core in ((0, 0), (1, 4)):
        h3 = results[core]["out"]                   # (DM, L) t2-canonical
        seq = h3.T.reshape(NCELL, NSITE, DM)        # v = c*NSITE + s
        out[b] = seq.transpose(1, 0, 2)
    return out


# ---------------------------------------------------------------------------
# cached PJRT runner (built once per process; repeat kernel() calls are fast)
# ---------------------------------------------------------------------------
import time

import jax
from jax.sharding import Mesh, PartitionSpec
from jax.experimental.shard_map import shard_map

from concourse.bass2jax import _bass_exec_p, install_neuronx_cc_hook, partition_id_tensor


class Runner:
    def __init__(self, nc, in_maps, n_cores=8):
        install_neuronx_cc_hook()
        self.n_cores = n_cores
        partition_name = nc.partition_id_tensor.name if nc.partition_id_tensor else None
        in_names, out_names, out_avals, zero_outs = [], [], [], []
        for alloc in nc.m.functions[0].allocations:
            if not isinstance(alloc, mybir.MemoryLocationSet):
                continue
            name = alloc.memorylocations[0].name
            if alloc.kind == "ExternalInput":
                if name != partition_name:
                    in_names.append(name)
            elif alloc.kind == "ExternalOutput":
                out_names.append(name)
                shape = tuple(alloc.tensor_shape)
                dtype = mybir.dt.np(alloc.dtype)
                out_avals.append(jax.core.ShapedArray(shape, dtype))
                zero_outs.append(np.zeros(shape, dtype))
        n_params = len(in_names)
        n_outs = len(out_avals)
        all_in_names = list(in_names) + out_names
        if partition_name is not None:
            all_in_names.append(partition_name)
        donate = tuple(range(n_params, n_params + n_outs))

        def _body(*args):
            operands = list(args)
            if partition_name is not None:
                operands.append(partition_id_tensor())
            outs = _bass_exec_p.bind(
                *operands,
                out_avals=tuple(out_avals),
                in_names=tuple(all_in_names),
                out_names=tuple(out_names),
                lowering_input_output_aliases=(),
                sim_require_finite=True,
                sim_require_nnan=True,
                nc=nc,
            )
            return tuple(outs)

        devices = jax.devices()[:n_cores]
        mesh = Mesh(np.asarray(devices), ("core",))
        in_specs = (PartitionSpec("core"),) * (n_params + n_outs)
        out_specs = (PartitionSpec("core"),) * n_outs
        self.f = jax.jit(
            shard_map(_body, mesh=mesh, in_specs=in_specs,
                      out_specs=out_specs, check_rep=False),
            donate_argnums=donate, keep_unused=True)
        self.in_names = in_names
        self.n_params = n_params
        self.sharding = jax.sharding.NamedSharding(mesh, PartitionSpec("core"))
        self.set_inputs(in_maps)
        zshapes = [(n_cores * z.shape[0], *z.shape[1:]) for z in zero_outs]
        zdt = [z.dtype for z in zero_outs]

        def _mkzeros():
            return tuple(jax.numpy.zeros(s, d) for s, d in zip(zshapes, zdt))

        self.mkzeros = jax.jit(_mkzeros, out_shardings=(self.sharding,) * n_outs)
        self.out_names = out_names
        self.out_avals = out_avals

    def set_inputs(self, in_maps):
        per_core = [[np.asarray(m[n]) for n in self.in_names] for m in in_maps]
        concat_in = [
            np.concatenate([per_core[c][i] for c in range(self.n_cores)], axis=0)
            for i in range(self.n_params)
        ]
        self.inputs_dev = [jax.device_put(a, self.sharding) for a in concat_in]

    def run(self):
        z = self.mkzeros()
        jax.block_until_ready(z)
        t0 = time.time()
        outs = self.f(*self.inputs_dev, *z)
        jax.block_until_ready(outs)
        dt = time.time() - t0
        return outs, dt

    def results(self, outs):
        res = []
        for c in range(self.n_cores):
            m = {}
            for i, name in enumerate(self.out_names):
                a = np.asarray(outs[i])
                m[name] = a.reshape(self.n_cores, *self.out_avals[i].shape)[c]
            res.append(m)
        return res

    def bench(self, warmup=2, iters=12):
        for _ in range(warmup):
            self.run()
        ts = []
        for _ in range(iters):
            _, dt = self.run()
            ts.append(dt)
        ts.sort()
        return ts[len(ts) // 2], ts[0]


_cache = {}


def _get_nc(nrep=1):
    if nrep not in _cache:
        _cache[nrep] = build_bass(nrep)
    return _cache[nrep]


_runner_cache = {}


def get_runner(inputs, nrep=1):
    key = nrep
    if key not in _runner_cache:
        _runner_cache[key] = Runner(_get_nc(nrep), make_in_maps(inputs), N_CORES)
    return _runner_cache[key]


def kernel(**inputs) -> np.ndarray:
    r = get_runner(inputs, 1)
    # refresh device inputs in case the caller passes different data
    in_maps = make_in_maps(inputs)
    r.set_inputs(in_maps)
    outs, _ = r.run()
    return postprocess(r.results(outs))
